# revision 8
# baseline (speedup 1.0000x reference)
"""Fused attention (QKV proj + softmax(QK^T/sqrt(d))V) for Trainium2,
SPMD over 8 NeuronCores -- "lambda-folded" formulation.

Key algebra: Q and K only appear through Q K^T = x (wq wk^T) x^T, and
attn @ V = (attn @ x) @ wv.  The host precomputes M = wq wk^T / sqrt(E)
(a weight-only transform, like serving-time weight fusion), so the
device does:

  TT  [D, SH] = M^T @ x_own^T          (phase 1)
  ST  [S, SH] = x_b   @ T^T, exp fused (phase 2)
  UT  [D, SH] = x_b^T @ expS^T         (phase 3)
  out [SH, E] = (U / den) @ wv         (phase 4)

12.9 GFLOP/core (vs 19.3 for the direct form with duplicated K/V) with
no collectives and no duplicated compute.  768 f=512 matmuls/core.

DMA discipline (the marginal-time bottleneck on this part):
  - x is loaded ONCE per rep as xT [D, S] (4MB); the [S, D] orientation
    needed by phase 3 is rebuilt on-chip with 128 DVE 32x32
    stream-transposes (~40us DVE, hidden under phase-1/2 PE work),
    saving 4MB/rep of DRAM traffic.
  - xT is double-buffered across reps (ping-pong), so rep k+1's 4MB
    prefetch has a ~2-rep window instead of the ~20us gap between
    phase-2's last read and the next rep.
  - queues: xT on SP (nc.sync), m+wv on Activation (nc.scalar), output
    stores on the gpsimd SWDGE queue -- an in-order HWDGE queue shared
    between outputs and inputs would block the next rep's prefetch
    behind this rep's compute.
  - output is bf16 (2MB instead of 4MB f32); the host upcasts.

Softmax denominator runs entirely off the PE: DVE adds accumulate
den_acc[p, i] = sum_t est[128t+p, i], gpsimd partition_all_reduce sums
the 128 partitions, 32x32 DVE stream-transposes move den onto
partitions, one reciprocal feeds the phase-4 per-partition scale.

Sharding: 8 shards = 4 batches x 2 query halves.  Inputs are passed
key-rolled (own query block first) so one uniform SPMD program serves
all cores; softmax/AV are permutation-invariant over keys.
"""

import time as time_mod
from contextlib import ExitStack

import numpy as np
import ml_dtypes

import concourse.bacc as bacc
import concourse.tile as tile
from concourse import mybir
from concourse import bass_isa
from concourse.bass_utils import run_bass_kernel_spmd

B, S, D, E = 4, 2048, 1024, 1024  # batch, seq, model dim, qkv dim
SH = S // 2                       # per-core query rows
P = 128
DT = D // P   # 8 d-tiles (model-dim tiles)
JT = S // P   # 16 key tiles
IT = SH // P  # 8 query-row blocks
BF16 = mybir.dt.bfloat16
F32 = mybir.dt.float32
NPBF16 = ml_dtypes.bfloat16

_compiled = {}


def _emit_v6(tc, ctx, xT_sb, xT, m, wv, out, rep=0):
    nc = tc.nc
    psum = ctx.enter_context(tc.tile_pool(name=f"p{rep}", bufs=4, space="PSUM"))
    outp = ctx.enter_context(tc.tile_pool(name=f"o{rep}", bufs=2))
    late = ctx.enter_context(tc.tile_pool(name=f"l{rep}", bufs=1))

    # late pool: tensors live to the end of this rep
    wv_sb = late.tile([P, DT * E], BF16, tag="wv", name="wv")
    ut_sb = [late.tile([P, SH], BF16, tag=f"ut{d}", name=f"ut{d}")
             for d in range(DT)]
    den_acc = late.tile([P, SH], F32, tag="den_acc", name="den_acc")
    trans = late.tile([P, 256], F32, tag="trans", name="trans")
    recip = late.tile([P, IT], F32, tag="recip", name="recip")

    # pool release must be LIFO: pm (freed after phase 1) opens after pb
    # (freed after phase 3)
    pb = ExitStack()  # xn_big, tt, est: freed after phase 3
    pm = ExitStack()  # m: freed after phase 1
    b = pb.enter_context(tc.tile_pool(name=f"b{rep}", bufs=1))
    mp = pm.enter_context(tc.tile_pool(name=f"mp{rep}", bufs=1))

    m_sb = mp.tile([P, DT * D], BF16, tag="m", name="m")
    tt_sb = [b.tile([P, SH], BF16, tag=f"tt{e}", name=f"tt{e}")
             for e in range(DT)]
    est_sb = [b.tile([P, SH], BF16, tag=f"est{j}", name=f"est{j}")
              for j in range(JT)]
    xn_big = b.tile([P, JT * D], BF16, tag="xn", name="xn")

    # input DMA: one merged transfer per tensor (per-transfer latency on
    # this part dwarfs bandwidth); xT on SP, m+wv on the Activation queue
    nc.sync.dma_start(xT_sb.rearrange("p (d s) -> p d s", d=DT),
                      xT.rearrange("(d p) s -> p d s", p=P))
    nc.scalar.dma_start(m_sb.rearrange("p (d e) -> p d e", d=DT),
                        m.rearrange("(d p) e -> p d e", p=P))
    nc.scalar.dma_start(wv_sb.rearrange("p (d e) -> p d e", d=DT),
                        wv.rearrange("(d p) e -> p d e", p=P))

    # on-chip transpose: xn_big[32a+r, j*1024+dt*128+32c+s] =
    # xT_sb[dt][32c+s, j*128+32a+r]  (DVE stream-transpose, 32x32 blocks)
    for dt in range(DT):
        for a in range(4):
            for c in range(4):
                src = xT_sb[32 * c:32 * (c + 1),
                            dt * S:(dt + 1) * S].rearrange(
                    "p (j a s) -> p j a s", j=JT, a=4, s=32)[:, :, a, :]
                dst = xn_big[32 * a:32 * (a + 1), :].rearrange(
                    "p (j d c s) -> p j d c s", j=JT, d=DT, c=4, s=32
                )[:, :, dt, c, :]
                nc.vector.transpose(dst, src)

    # Phase 1: TT[e,:] = sum_d m[d, e-block].T @ xT[d, :SH].
    # d-outer waves of 4 e-groups so the PE tracks DMA arrival order on
    # the first rep (every e-group needs all 8 m/xT d-tiles).
    for w in range(2):
        es = range(w * 4, w * 4 + 4)
        ps = {e: psum.tile([P, SH], F32, tag="mm", name=f"tt_ps{e}") for e in es}
        for d in range(DT):
            for e in es:
                lhsT = m_sb[:, d * D + e * P:d * D + (e + 1) * P]
                for h in (0, 512):
                    nc.tensor.matmul(ps[e][:, h:h + 512], lhsT,
                                     xT_sb[:, d * S + h:d * S + h + 512],
                                     start=(d == 0), stop=(d == DT - 1))
        for e in es:
            nc.scalar.copy(tt_sb[e], ps[e])

    pm.close()  # m dead; next rep's m can load here

    # Phase 2: ST[j-block, :] = sum_e xT[e, j-block].T @ TT[e, :]; exp fused.
    # DVE accumulates den_acc[p, i] = sum_t est[128t+p, i] as tiles land.
    for j in range(JT):
        ps = psum.tile([P, SH], F32, tag="mm", name="st_ps")
        for e in range(DT):
            lhsT = xT_sb[:, e * S + j * P:e * S + (j + 1) * P]
            for h in (0, 512):
                nc.tensor.matmul(ps[:, h:h + 512], lhsT, tt_sb[e][:, h:h + 512],
                                 start=(e == 0), stop=(e == DT - 1))
        nc.scalar.activation(est_sb[j], ps, mybir.ActivationFunctionType.Exp)
        if j == 0:
            nc.vector.tensor_copy(den_acc, est_sb[j])
        else:
            nc.vector.tensor_tensor(den_acc, den_acc, est_sb[j],
                                    mybir.AluOpType.add)

    # den: sum the 128 partitions, then 32x32 stream-transposes put den[i]
    # on partition i%128 (trans[p, 32*blk + a] = den[128*blk + p]).
    nc.gpsimd.partition_all_reduce(den_acc, den_acc, P, bass_isa.ReduceOp.add)
    for k in range(SH // 32):
        sub, blk = k % 4, k // 4
        nc.vector.transpose(trans[sub * 32:(sub + 1) * 32,
                                  blk * 32:(blk + 1) * 32],
                            den_acc[0:32, k * 32:(k + 1) * 32])
    for blk in range(IT):
        nc.vector.reciprocal(recip[:, blk:blk + 1],
                             trans[:, blk * 32:blk * 32 + 1])

    # Phase 3: UT[d-block, :] = sum_j xn[j, d-block].T @ est[j, :]
    for dblk in range(DT):
        ps = psum.tile([P, SH], F32, tag="mm", name="ut_ps")
        for j in range(JT):
            lhsT = xn_big[:, j * D + dblk * P:j * D + (dblk + 1) * P]
            for h in (0, 512):
                nc.tensor.matmul(ps[:, h:h + 512], lhsT, est_sb[j][:, h:h + 512],
                                 start=(j == 0), stop=(j == JT - 1))
        nc.scalar.copy(ut_sb[dblk], ps)

    pb.close()  # xn_big, tt, est dead

    # Phase 4: out[i-block, :] = (sum_d UT[d, i-block].T @ wv[d, :]) * recip
    for ib in range(IT):
        ps = psum.tile([P, E], F32, tag="mm", name="av_ps")
        for d in range(DT):
            lhsT = ut_sb[d][:, ib * P:(ib + 1) * P]
            for h in (0, 512):
                nc.tensor.matmul(ps[:, h:h + 512], lhsT,
                                 wv_sb[:, d * E + h:d * E + h + 512],
                                 start=(d == 0), stop=(d == DT - 1))
        o = outp.tile([P, E], BF16, tag="o")
        nc.vector.tensor_scalar_mul(o, ps, recip[:, ib:ib + 1])
        # outputs go out on the gpsimd SWDGE queue: a shared in-order
        # HWDGE queue would block the next rep's input prefetch behind
        # this rep's phase-4 compute
        nc.gpsimd.dma_start(out[ib * P:(ib + 1) * P, :], o)


def _build(repeats=1):
    key = ("v7", repeats)
    if key not in _compiled:
        nc = bacc.Bacc("TRN2", target_bir_lowering=False, debug=False,
                       num_devices=8)
        xT = nc.dram_tensor("xT", [D, S], BF16, kind="ExternalInput").ap()
        m = nc.dram_tensor("m", [D, D], BF16, kind="ExternalInput").ap()
        wv = nc.dram_tensor("wv", [D, E], BF16, kind="ExternalInput").ap()
        out = nc.dram_tensor("out", [SH, E], BF16, kind="ExternalOutput").ap()
        with tile.TileContext(nc) as tc:
            with ExitStack() as ctx0:
                # xT is double-buffered across reps (ping-pong) so the next
                # rep's 4MB load is gated on the rep-before-last, not on
                # this rep's phase-2 completion.
                xp = ctx0.enter_context(tc.tile_pool(name="xp", bufs=1))
                xbufs = [
                    xp.tile([P, DT * S], BF16, tag=f"x{par}",
                            name=f"x{par}") for par in range(2)
                ]
                # PE warm-up during the initial DMA fill keeps the HAM
                # clock-gate warm
                warm_src = xp.tile([P, 512], BF16, tag="warm_src",
                                   name="warm_src")
                tc.nc.vector.memset(warm_src, 0.0)
                with tc.tile_pool(name="wp", bufs=1, space="PSUM") as wps:
                    warm_ps = wps.tile([P, 512], F32, tag="w", name="warm_ps")
                    for _ in range(4):
                        tc.nc.tensor.matmul(warm_ps, warm_src[:, 0:P],
                                            warm_src)
                for rep in range(repeats):
                    with ExitStack() as ctx:
                        _emit_v6(tc, ctx, xbufs[rep % 2], xT, m, wv, out,
                                 rep=rep)
        nc.compile()
        _compiled[key] = nc
    return _compiled[key]


def _make_in_maps(x, wq, wk, wv):
    wq32 = np.asarray(wq, np.float32)
    wk32 = np.asarray(wk, np.float32)
    m_np = np.ascontiguousarray(
        (wq32 @ wk32.T) * np.float32(1.0 / np.sqrt(E))).astype(NPBF16)
    wv_bf = np.ascontiguousarray(wv).astype(NPBF16)
    in_maps = []
    for c in range(8):
        b, h = c // 2, c % 2
        # roll keys so this core's query block is always rows/cols 0:SH
        xr = np.concatenate([x[b, h * SH:], x[b, :h * SH]], axis=0)
        in_maps.append({
            "xT": np.ascontiguousarray(xr.T).astype(NPBF16),
            "m": m_np,
            "wv": wv_bf,
        })
    return in_maps


def kernel(x, wq, wk, wv, _trace=False):
    x = np.asarray(x, dtype=np.float32)
    nc = _build()
    in_maps = _make_in_maps(x, np.asarray(wq), np.asarray(wk), np.asarray(wv))
    try:
        res = run_bass_kernel_spmd(nc, in_maps, core_ids=list(range(8)),
                                   trace=_trace)
    except Exception:
        # transient NRT_EXEC_UNIT_UNRECOVERABLE wedges have been observed to
        # clear on a fresh attempt
        time_mod.sleep(5)
        res = run_bass_kernel_spmd(nc, in_maps, core_ids=list(range(8)),
                                   trace=_trace)
    full = np.empty((B, S, E), np.float32)
    for c in range(8):
        b, h = c // 2, c % 2
        full[b, h * SH:(h + 1) * SH] = res.results[c]["out"]
    if _trace:
        kernel.last_results = res
    return full
